# revision 18
# baseline (speedup 1.0000x reference)
"""Trainium2 Bass kernel for nn_Criterion_49237505081886.

reference semantics: the torch loop overwrites `loss` each iteration, so the
returned scalar depends ONLY on the last batch row:

    S    = sum_j (y[-1,j] - mu[-1,j])^2 / sigma[-1,j] + log(sigma[-1,j])
    loss = 0.5 * (S + NT*log(2*pi)) / (NT * BS)

The kernel ships just the last row (3 x 2048 f32 = 24 KiB, packed into one
[128, 48] DMA), computes everything on-device, and DMAs back one f32 scalar:

  DVE: diff = y-mu; r = 1/sigma; w = diff*r;
       stt: acc[p,0] = sum_f w*diff              (fused mul+reduce)
  ACT: acc[p,1] = sum_f Ln(sigma)               (table preloaded via dummy op)
  DVE: acc[p,2] = C (memset, early)             (C folds in NT*log(2pi))
  PE : ps[1,3] = svec.T @ acc, svec = 2^-24 = 0.5/(NT*BS)  (exact pow2 scale)
  DVE: loss = reduce_add(ps[1,3]);  SP: DMA out.

Runs SPMD-replicated on all 8 cores; core 0's scalar is the result.
"""
import sys

if "/opt/trn_rl_repo" not in sys.path:  # harness runs from a bare directory
    sys.path.append("/opt/trn_rl_repo")

import numpy as np

LOG_2PI = 1.8378770664093453
BS, NT = 4096, 2048
P, F = 128, 16  # 2048 = 128 * 16
N_CORES = 8

SCALE = 0.5 / (NT * BS)  # == 2**-24, exact in f32
# Constant column for the matmul: SCALE * P * C_INIT == 0.5*NT*log(2pi)/(NT*BS)
C_INIT = (0.5 * LOG_2PI / BS) / (P * SCALE)

# The NEFF runtime drains all DGE queues before completing an execution, so
# the kernel does not need to busy-wait on the output DMA's semaphore; the
# completion overlaps the block-exit barrier. Toggle to re-add the wait.
FINAL_DMA_WAIT = False

_CACHE = {}


def build_nc():
    import concourse.bass as bass
    import concourse.mybir as mybir

    f32 = mybir.dt.float32
    Act = mybir.ActivationFunctionType
    Alu = mybir.AluOpType

    nc = bass.Bass()
    packed_d = nc.declare_dram_parameter("packed", [P, 3 * F], f32, isOutput=False)
    loss_d = nc.declare_dram_parameter("loss", [1, 1], f32, isOutput=True)

    with (
        nc.sbuf_tensor("packed_sb", [P, 3 * F], f32) as packed_sb,
        nc.sbuf_tensor("diff", [P, F], f32) as diff,
        nc.sbuf_tensor("recip", [P, F], f32) as recip,
        nc.sbuf_tensor("w", [P, F], f32) as w,
        nc.sbuf_tensor("scr", [P, F], f32) as scr,
        nc.sbuf_tensor("lnsg", [P, F], f32) as lnsg,
        nc.sbuf_tensor("acc2", [P, 3], f32) as acc2,
        nc.sbuf_tensor("svec", [P, 1], f32) as svec,
        nc.sbuf_tensor("dum", [1, 1], f32) as dum,
        nc.sbuf_tensor("loss_sb", [1, 1], f32) as loss_sb,
        nc.psum_tensor("ps", [1, 3], f32) as ps,
        nc.semaphore("dma_sem") as dma_sem,
        nc.semaphore("act_sem") as act_sem,
        nc.semaphore("vec_sem") as vec_sem,
        nc.semaphore("mm_sem") as mm_sem,
        nc.Block() as block,
    ):
        mu_sb = packed_sb[:, 0:F]
        sg_sb = packed_sb[:, F : 2 * F]
        ty_sb = packed_sb[:, 2 * F : 3 * F]

        @block.sync
        def _(sync):
            sync.dma_start(packed_sb[:], packed_d[:]).then_inc(dma_sem, 16)
            sync.wait_ge(vec_sem, 7)
            sync.dma_start(
                loss_d[:], loss_sb[:], single_packet=True
            ).then_inc(dma_sem, 16)
            if FINAL_DMA_WAIT:
                sync.wait_ge(dma_sem, 32)

        @block.vector
        def _(vector):
            vector.memset(svec[:], SCALE).then_inc(vec_sem, 1)
            vector.memset(acc2[:, 2:3], C_INIT).then_inc(vec_sem, 1)
            vector.wait_ge(dma_sem, 16)
            vector.reciprocal(recip[:], sg_sb).then_inc(vec_sem, 1)
            vector.tensor_sub(diff[:], ty_sb, mu_sb).then_inc(vec_sem, 1)
            vector.wait_ge(vec_sem, 4)
            vector.tensor_mul(w[:], diff[:], recip[:]).then_inc(vec_sem, 1)
            vector.wait_ge(vec_sem, 5)
            vector.scalar_tensor_tensor(
                scr[:],
                w[:],
                1.0,
                diff[:],
                op0=Alu.mult,
                op1=Alu.mult,
                accum_out=acc2[:, 0:1],
            ).then_inc(vec_sem, 1)
            vector.wait_ge(mm_sem, 1)
            vector.tensor_reduce(
                loss_sb[:], ps[:], axis=mybir.AxisListType.X, op=Alu.add
            ).then_inc(vec_sem, 1)

        @block.scalar
        def _(scalar):
            # Dummy Ln on garbage (scale=0 kills the read) to pull the ACT
            # table load off the critical path, during the DMA wait.
            scalar.activation(dum[:], dum[:], Act.Ln, scale=0.0, bias=1.0).then_inc(
                act_sem, 1
            )
            scalar.wait_ge(dma_sem, 16)
            scalar.activation(
                lnsg[:], sg_sb, Act.Ln, accum_out=acc2[:, 1:2]
            ).then_inc(act_sem, 1)

        @block.tensor
        def _(tensor):
            tensor.wait_ge(vec_sem, 6)
            tensor.wait_ge(act_sem, 2)
            tensor.matmul(ps[:], svec[:], acc2[:], start=True, stop=True).then_inc(
                mm_sem, 1
            )

    return nc


def _get_nc():
    if "nc" not in _CACHE:
        _CACHE["nc"] = build_nc()
    return _CACHE["nc"]


def make_in_maps(mu, sigma, target_y):
    mu = np.asarray(mu, dtype=np.float32)
    sigma = np.asarray(sigma, dtype=np.float32)
    target_y = np.asarray(target_y, dtype=np.float32)
    packed = np.concatenate(
        [
            np.asarray(mu[-1]).reshape(P, F),
            np.asarray(sigma[-1]).reshape(P, F),
            np.asarray(target_y[-1]).reshape(P, F),
        ],
        axis=1,
    )
    packed = np.ascontiguousarray(packed)
    in_map = {"packed": packed}
    return [in_map for _ in range(N_CORES)]


def kernel(mu, sigma, target_y):
    from concourse.bass_utils import run_bass_kernel_spmd

    in_maps = make_in_maps(mu, sigma, target_y)
    res = run_bass_kernel_spmd(_get_nc(), in_maps, list(range(N_CORES))).results
    return np.asarray(res[0]["loss"], dtype=np.float32).reshape(())


# revision 20
# speedup vs baseline: 1.0451x; 1.0451x over previous
"""Trainium2 Bass kernel for nn_Criterion_49237505081886.

reference semantics: the torch loop overwrites `loss` each iteration, so the
returned scalar depends ONLY on the last batch row:

    S    = sum_j (y[-1,j] - mu[-1,j])^2 / sigma[-1,j] + log(sigma[-1,j])
    loss = 0.5 * (S + NT*log(2*pi)) / (NT * BS)

The kernel ships just the last row (3 x 2048 f32 = 24 KiB, packed into one
[128, 48] DMA), computes everything on-device, and DMAs back one f32 scalar:

  DVE: diff = y-mu; r = 1/sigma; w = diff*r;
       stt: acc[p,0] = sum_f w*diff              (fused mul+reduce)
  ACT: acc[p,1] = sum_f Ln(sigma)               (table preloaded via dummy op)
  DVE: acc[p,2] = C (memset, early)             (C folds in NT*log(2pi))
  PE : ps[1,3] = svec.T @ acc, svec = 2^-24 = 0.5/(NT*BS)  (exact pow2 scale)
  DVE: loss = reduce_add(ps[1,3]);  SP: DMA out.

Runs SPMD-replicated on all 8 cores; core 0's scalar is the result.
"""
import sys

if "/opt/trn_rl_repo" not in sys.path:  # harness runs from a bare directory
    sys.path.append("/opt/trn_rl_repo")

import numpy as np

LOG_2PI = 1.8378770664093453
BS, NT = 4096, 2048
P, F = 128, 16  # 2048 = 128 * 16
N_CORES = 8

SCALE = 0.5 / (NT * BS)  # == 2**-24, exact in f32
# Constant column for the matmul: SCALE * P * C_INIT == 0.5*NT*log(2pi)/(NT*BS)
C_INIT = (0.5 * LOG_2PI / BS) / (P * SCALE)

# The NEFF runtime drains all DGE queues before completing an execution, so
# the kernel does not need to busy-wait on the output DMA's semaphore; the
# completion overlaps the block-exit barrier. Toggle to re-add the wait.
FINAL_DMA_WAIT = False

_CACHE = {}


def build_nc():
    import concourse.bass as bass
    import concourse.mybir as mybir

    f32 = mybir.dt.float32
    Act = mybir.ActivationFunctionType
    Alu = mybir.AluOpType

    nc = bass.Bass()
    packed_d = nc.declare_dram_parameter("packed", [P, 3 * F], f32, isOutput=False)
    loss_d = nc.declare_dram_parameter("loss", [1, 1], f32, isOutput=True)

    with (
        nc.sbuf_tensor("packed_sb", [P, 3 * F], f32) as packed_sb,
        nc.sbuf_tensor("diff", [P, F], f32) as diff,
        nc.sbuf_tensor("recip", [P, F], f32) as recip,
        nc.sbuf_tensor("w", [P, F], f32) as w,
        nc.sbuf_tensor("scr", [P, F], f32) as scr,
        nc.sbuf_tensor("lnsg", [P, F], f32) as lnsg,
        nc.sbuf_tensor("acc2", [P, 3], f32) as acc2,
        nc.sbuf_tensor("svec", [P, 1], f32) as svec,
        nc.sbuf_tensor("dum", [1, 1], f32) as dum,
        nc.sbuf_tensor("loss_sb", [1, 1], f32) as loss_sb,
        nc.psum_tensor("ps", [1, 3], f32) as ps,
        nc.semaphore("dma_sem") as dma_sem,
        nc.semaphore("act_sem") as act_sem,
        nc.semaphore("vec_sem") as vec_sem,
        nc.semaphore("mm_sem") as mm_sem,
        nc.Block() as block,
    ):
        mu_sb = packed_sb[:, 0:F]
        sg_sb = packed_sb[:, F : 2 * F]
        ty_sb = packed_sb[:, 2 * F : 3 * F]

        @block.sync
        def _(sync):
            sync.dma_start(packed_sb[:], packed_d[:]).then_inc(dma_sem, 16)
            sync.wait_ge(vec_sem, 7)
            sync.dma_start(
                loss_d[:], loss_sb[:], single_packet=True
            ).then_inc(dma_sem, 16)
            if FINAL_DMA_WAIT:
                sync.wait_ge(dma_sem, 32)

        @block.vector
        def _(vector):
            vector.memset(svec[:], SCALE).then_inc(vec_sem, 1)
            vector.memset(acc2[:, 2:3], C_INIT).then_inc(vec_sem, 1)
            vector.wait_ge(dma_sem, 16)
            vector.reciprocal(recip[:], sg_sb).then_inc(vec_sem, 1)
            vector.tensor_sub(diff[:], ty_sb, mu_sb).then_inc(vec_sem, 1)
            vector.wait_ge(vec_sem, 4)
            vector.tensor_mul(w[:], diff[:], recip[:]).then_inc(vec_sem, 1)
            vector.wait_ge(vec_sem, 5)
            vector.wait_ge(act_sem, 2)
            vector.scalar_tensor_tensor(
                scr[:],
                w[:],
                1.0,
                diff[:],
                op0=Alu.mult,
                op1=Alu.mult,
                accum_out=acc2[:, 0:1],
            ).then_inc(vec_sem, 1)
            vector.wait_ge(mm_sem, 1)
            vector.tensor_reduce(
                loss_sb[:], ps[:], axis=mybir.AxisListType.X, op=Alu.add
            ).then_inc(vec_sem, 1)

        @block.scalar
        def _(scalar):
            # Dummy Ln on garbage (scale=0 kills the read) to pull the ACT
            # table load off the critical path, during the DMA wait.
            scalar.activation(dum[:], dum[:], Act.Ln, scale=0.0, bias=1.0).then_inc(
                act_sem, 1
            )
            scalar.wait_ge(dma_sem, 16)
            scalar.activation(
                lnsg[:], sg_sb, Act.Ln, accum_out=acc2[:, 1:2]
            ).then_inc(act_sem, 1)

        @block.tensor
        def _(tensor):
            # acc2 col1 (ACT) is transitively covered: stt1 gates on act_sem.
            tensor.wait_ge(vec_sem, 6)
            tensor.matmul(ps[:], svec[:], acc2[:], start=True, stop=True).then_inc(
                mm_sem, 1
            )

    return nc


def _get_nc():
    if "nc" not in _CACHE:
        _CACHE["nc"] = build_nc()
    return _CACHE["nc"]


def make_in_maps(mu, sigma, target_y):
    mu = np.asarray(mu, dtype=np.float32)
    sigma = np.asarray(sigma, dtype=np.float32)
    target_y = np.asarray(target_y, dtype=np.float32)
    packed = np.concatenate(
        [
            np.asarray(mu[-1]).reshape(P, F),
            np.asarray(sigma[-1]).reshape(P, F),
            np.asarray(target_y[-1]).reshape(P, F),
        ],
        axis=1,
    )
    packed = np.ascontiguousarray(packed)
    in_map = {"packed": packed}
    return [in_map for _ in range(N_CORES)]


def kernel(mu, sigma, target_y):
    from concourse.bass_utils import run_bass_kernel_spmd

    in_maps = make_in_maps(mu, sigma, target_y)
    res = run_bass_kernel_spmd(_get_nc(), in_maps, list(range(N_CORES))).results
    return np.asarray(res[0]["loss"], dtype=np.float32).reshape(())
